# revision 33
# baseline (speedup 1.0000x reference)
"""Single-head causal attention (B=4, T=2048, C=1024) on 8 trn2 NeuronCores.

Sharding: 8 shards = (batch b in 0..3) x (query interleave h in 0..1), same
balanced interleaved-256 query split as the fp32r baseline: core h of a pair
takes global 256-row query blocks {2*bg+h}, so the causal triangle is
balanced across the pair. One SPMD instruction stream; all per-core
variation is data (gathered x slices + three [128,128] mask tiles).

All matmuls run as compensated fp8e4m3 DoubleRow pairs. Each operand is
decomposed as v = hi + lo (hi = fp8(v), lo = fp8(v - hi)); a product
x*w = xh*wh + (xh*wl + xl*wh) keeps ~bf16 accuracy (residual ~0.1%) while
DoubleRow processes TWO 128-deep contraction planes per instruction at 0.5
cycles/row -- 4x the fp32r/bf16 rate, so the compensated triple costs 0.75x
of the bf16-equivalent. Host-side tensors (x, weights) are decomposed on
the host; device-computed tensors (q, k, v, exp-scores, attention out) are
decomposed with a cast + subtract pass (Act/DVE/Pool engines, all far under
the PE roofline).

Algebraic folds:
  - k bias dropped entirely: softmax over kv positions is invariant to
    per-query constants, and (q+bq).(k+bk) - (q+bq).k is constant per row.
  - q bias folded into the exp: s_ij = q~_i.k_j + (bq~.k_j), the second
    term is per-kv-position, computed on device as a tiny N=1 DoubleRow
    matmul chain (bqk), and applied as the Exp activation bias together
    with -ln(32).
  - exp(s)/32 stored instead of exp(s) so fp8's 240 max is never hit
    (scores ~N(0,1); the 1/32 cancels between att@V and the rowsum).
  - v bias folded into the output bias (beff = b_proj + w_proj @ b_v).
  - 1/sqrt(C) folded into wq/bq host-side.

fp8 halves SBUF, so unlike the baseline there is NO DRAM spill of kv half 1
(phases C/skT/sV are gone): kT/V for all 2048 kv positions, qT, AT and the
fp32 O accumulator are all resident.

Comp-plane storage convention (so compensated cross terms pair cleanly):
"moving-side" tensors (x, qT, AT, Opair, bq) store (hi, lo); "stationary
side" (wk, wq, wv, wp, kT, V) store (lo, hi). A cross op then reads
lhsT[:, 0:2] x rhs[:, 0:2] = wl*xh + wh*xl directly; hi*hi ops index plane
1 of the stationary and plane 0 of the moving tensor, pairing adjacent
contraction subtiles instead.
"""

import sys

sys.path.insert(0, "/opt/trn_rl_repo")

import numpy as np

import concourse.bass as bass
import concourse.tile as tile
from concourse import mybir
from concourse.vector_clock import ScopedClock

FP = mybir.dt.float32
BF = mybir.dt.bfloat16
F8 = mybir.dt.float8e4
AF = mybir.ActivationFunctionType
DR = mybir.MatmulPerfMode.DoubleRow

P = 128
C = 1024  # embed dim
H = 1024  # query rows per core
TKV = 2048  # kv length
NT = C // P  # 8 c-subtiles
NKV = TKV // P  # 16 kv-subtiles
NEG = -1.0e9
ASCALE = 32.0  # exp(s)/ASCALE stored in fp8

_MAX_WAITS = 1


class _TC(tile.TileContext):
    """TileContext whose tail drain puts its global-clock waits on a nop
    (walrus rejects multi-wait Drain); excess waits are split by
    _split_waits() afterwards."""

    def _drain_and_barrier(self, tick_clock, wait_clock):
        nop_inst = self.nc.sync.nop(nofuse=True, hint="pre_drain_waits")
        wait_clock.add_sem_waits(
            nop_inst.ins, ScopedClock({None: tick_clock.global_clock})
        )
        self.nc.sync.drain()
        self.nc.all_engine_barrier()
        assert self.sems is not None
        popped = self.nc._tile_sem_poison_stack.pop()
        assert popped is self._sem_poison
        self.nc.clear_and_free_semaphores(list(self.sems.allocated().values()))
        self.nc.all_engine_barrier()


def _split_waits(nc, max_waits=_MAX_WAITS):
    """The walrus shipped here rejects instructions carrying more than
    `max_waits` sync waits. Move excess waits onto injected nops placed
    immediately before the instruction on the same engine."""
    import copy

    template = nc.sync.nop(nofuse=True, hint="waitsplit_template").ins
    counter = [0]

    def make_nop(engine, waits):
        nop = copy.deepcopy(template)
        counter[0] += 1
        nop.name = f"I-wsplit-{counter[0]}"
        nop.engine = engine
        nop.sync_info = mybir.SyncInfo(on_wait=list(waits), on_update=[])
        return nop

    f = nc.m.functions[0]
    for bb in f.blocks:
        insts = bb.instructions
        if not any(
            i.sync_info and i.sync_info.on_wait and len(i.sync_info.on_wait) > max_waits
            for i in insts
        ):
            continue
        newlist = []
        for inst in insts:
            si = inst.sync_info
            if si and si.on_wait and len(si.on_wait) > max_waits:
                if inst.name == template.name:
                    newlist.append(inst)
                    continue
                waits = list(si.on_wait)
                del si.on_wait[max_waits:]
                rest = waits[max_waits:]
                while rest:
                    newlist.append(make_nop(inst.engine, rest[:max_waits]))
                    rest = rest[max_waits:]
            newlist.append(inst)
        bb.instructions[:] = newlist


# Causal structure for the interleaved-256 query sharding, over 16 kv
# 128-subtiles. Query slots bg=0..3 hold global 256-row blocks g=2*bg+h.
# For kv subtile s, valid query cols start at LO16[s]*128; mask tiles
# (data-encoded per core) are added at the listed 128-col block positions.
LO16 = [0, 0, 0, 1, 2, 2, 2, 3, 4, 4, 4, 5, 6, 6, 6, 7]
_MASKS8 = [
    [(0, 0)],            # (128-block, mask index) ; 0=m1d 1=m1f 2=m2d
    [(0, 1), (1, 0)],
    [(0, 2), (1, 1)],
    [(1, 2)],
    [(2, 0)],
    [(2, 1), (3, 0)],
    [(2, 2), (3, 1)],
    [(3, 2)],
]
MASKS16 = [
    [((s // 8) * 4 + off, mi) for off, mi in _MASKS8[s % 8]] for s in range(16)
]
# pair-aligned lo (attv pairs kv subtiles (2p, 2p+1))
LOP16 = [LO16[s] - (LO16[s] % 2) for s in range(16)]


def _chunks512(lo, hi):
    """Split [lo, hi) at absolute multiples of 512."""
    out = []
    while lo < hi:
        ce = min((lo // 512 + 1) * 512, hi)
        out.append((lo, ce))
        lo = ce
    return out


def _build_nc():
    nc = bass.Bass("TRN2", target_bir_lowering=False, debug=False)

    xq_in = nc.dram_tensor("xq_in", [P, 2, NT, H], F8, kind="ExternalInput").ap()
    xoA_in = nc.dram_tensor("xoA_in", [P, 2, NT, 256], F8, kind="ExternalInput").ap()
    xoB_in = nc.dram_tensor("xoB_in", [P, 2, NT, 256], F8, kind="ExternalInput").ap()
    xoC_in = nc.dram_tensor("xoC_in", [P, 2, NT, 512], F8, kind="ExternalInput").ap()
    xx_in = nc.dram_tensor("xx_in", [P, 2, NT, H], F8, kind="ExternalInput").ap()
    wk_in = nc.dram_tensor("wk_in", [NT, P, 2, NT, P], F8, kind="ExternalInput").ap()
    wq_in = nc.dram_tensor("wq_in", [P, 2, NT, C], F8, kind="ExternalInput").ap()
    wv_in = nc.dram_tensor("wv_in", [P, 2, NT, C], F8, kind="ExternalInput").ap()
    wp_in = nc.dram_tensor("wp_in", [P, 2, NT, C], F8, kind="ExternalInput").ap()
    bqp_in = nc.dram_tensor("bqp_in", [P, 2, NT, 1], F8, kind="ExternalInput").ap()
    ones_in = nc.dram_tensor("ones_in", [P, 2, P], F8, kind="ExternalInput").ap()
    masks_in = nc.dram_tensor("masks_in", [P, 3, P], FP, kind="ExternalInput").ap()
    # beff (8 cols) | -ln(ASCALE) | -1e9 sliver-kill | 0 (boosted-exp bias)
    bias_in = nc.dram_tensor("bias_in", [P, NT + 3], FP, kind="ExternalInput").ap()
    # output, (o2-tile, chunk)-major, bf16; host reassembles + upcasts
    yT = nc.dram_tensor("yT", [NT * 2 * P, 512], BF, kind="ExternalOutput").ap()

    with _TC(nc) as tc:
        with (
            tc.tile_pool(name="misc", bufs=1) as misc,
            tc.tile_pool(name="kqv", bufs=1) as kqv,
            tc.tile_pool(name="psum", bufs=6, space="PSUM") as pp,
        ):
            ones_sb = misc.tile([P, 2, P], F8, tag="ones")
            masks = misc.tile([P, 3, P], FP, tag="masks")
            bias_sb = misc.tile([P, NT + 3], FP, tag="bias")
            bqp = misc.tile([P, 2, NT, 1], F8, tag="bqp")
            bqk_sb = misc.tile([P, NKV], FP, tag="bqk")
            bqk_sb2 = misc.tile([P, NKV], FP, tag="bqk2")

            # persistent fp8 pair tensors (comp order noted)
            kT = kqv.tile([P, 2, NT, TKV], F8, tag="kT")   # (lo, hi)
            qT = kqv.tile([P, 2, NT, H], F8, tag="qT")     # (hi, lo)
            V = kqv.tile([P, 2, NKV, C], F8, tag="V")      # (lo, hi)
            wp = kqv.tile([P, 2, NT, C], F8, tag="wp")     # (lo, hi)

            # =============================================================
            # Phase A: projections
            # =============================================================
            with tc.tile_pool(name="xw", bufs=1) as xw:
                # x half-0 split into 3 tiles, wk into 8 per-ot tiles:
                # tile deps are per-tile, so fine-grained tiles let the first
                # kproj group start as soon as its own DMAs land
                xoA = xw.tile([P, 2, NT, 256], F8, tag="xoA")
                xoB = xw.tile([P, 2, NT, 256], F8, tag="xoB")
                xoC = xw.tile([P, 2, NT, 512], F8, tag="xoC")
                xx = xw.tile([P, 2, NT, H], F8, tag="xx")
                xq = xw.tile([P, 2, NT, H], F8, tag="xq")
                wk = [
                    xw.tile([P, 2, NT, P], F8, tag=f"wk{ot}", name=f"wk{ot}")
                    for ot in range(NT)
                ]
                wq = xw.tile([P, 2, NT, C], F8, tag="wq")
                wv = xw.tile([P, 2, NT, C], F8, tag="wv")

                # fine-grained first loads so kproj starts ASAP
                nc.sync.dma_start(wk[0][:], wk_in[0])
                nc.sync.dma_start(xoA[:], xoA_in[:])
                for ot in range(1, NT):
                    nc.sync.dma_start(wk[ot][:], wk_in[ot])
                nc.sync.dma_start(xoB[:], xoB_in[:])
                nc.sync.dma_start(xoC[:], xoC_in[:])
                nc.sync.dma_start(xx[:], xx_in[:])
                nc.sync.dma_start(ones_sb[:], ones_in[:])
                nc.sync.dma_start(masks[:], masks_in[:])
                nc.sync.dma_start(bias_sb[:], bias_in[:])
                nc.sync.dma_start(bqp[:], bqp_in[:])
                nc.sync.dma_start(xq[:], xq_in[:])
                nc.sync.dma_start(wq[:], wq_in[:])
                nc.sync.dma_start(wv[:], wv_in[:])
                nc.sync.dma_start(wp[:], wp_in[:])

                # PE p-state warmup on a memset tile: no DMA dependency, so
                # the ramp completes while the first x/w transfers land
                wsrc = xw.tile([P, 2, P], F8, tag="wsrc")
                nc.vector.memset(wsrc[:], 1.0)
                wps = pp.tile([P, 512], FP, tag="ps", name="wps")
                for _ in range(40):
                    nc.tensor.matmul(
                        wps[:, 0:P],
                        lhsT=wsrc[:],
                        rhs=wsrc[:, :, :],
                        start=True,
                        stop=True,
                        perf_mode=DR,
                        skip_group_check=True,
                    )

                # (x tile, local col range, global kv base) pieces
                kchunks = [
                    (xoA, 0, 256, 0),
                    (xoB, 0, 256, 256),
                    (xoC, 0, 512, 512),
                    (xx, 0, 512, 1024),
                    (xx, 512, 1024, 1536),
                ]
                # vproj: token-tile tt of half -> (x tile, local col base)
                def vtile(half, tt):
                    if half == 1:
                        return xx, tt * P
                    if tt < 2:
                        return xoA, tt * P
                    if tt < 4:
                        return xoB, (tt - 2) * P
                    return xoC, (tt - 4) * P

                def mm12(ps, w, x, osl, cs, ce, n_start=True, n_stop=True):
                    """12-op compensated group: out[osl, cs:ce] += w.T @ x.
                    w stored (lo,hi), x stored (hi,lo); contraction over all
                    NT c-subtiles."""
                    first = [n_start]
                    for t in range(NT // 2):
                        nc.tensor.matmul(
                            ps[:, : ce - cs],
                            lhsT=w[:, 1, 2 * t : 2 * t + 2, osl],
                            rhs=x[:, 0, 2 * t : 2 * t + 2, cs:ce],
                            start=first[0],
                            stop=False,
                            perf_mode=DR,
                        )
                        first[0] = False
                    for ct in range(NT):
                        nc.tensor.matmul(
                            ps[:, : ce - cs],
                            lhsT=w[:, 0:2, ct, osl],
                            rhs=x[:, 0:2, ct, cs:ce],
                            start=False,
                            stop=(n_stop and ct == NT - 1),
                            perf_mode=DR,
                        )

                # ---- k projection (no bias; softmax-invariant) ----------
                sc = tc.nc.named_scope("A_k"); sc.__enter__()
                for xh, cs, ce, gb in kchunks:
                    for ot in range(NT):
                        ps = pp.tile([P, 512], FP, tag="ps", name=f"psk{gb}_{ot}")
                        mm12(ps, wk[ot], xh, slice(0, P), cs, ce)
                        g0, g1 = gb, gb + (ce - cs)
                        nc.scalar.activation(
                            kT[:, 1, ot, g0:g1], ps[:, : ce - cs], AF.Identity
                        )
                        nc.vector.tensor_sub(
                            kT[:, 0, ot, g0:g1],
                            ps[:, : ce - cs],
                            kT[:, 1, ot, g0:g1],
                        )
                sc.__exit__(None, None, None)

                # ---- bqk: per-kv-position q-bias term (bq~ . k_j) -------
                sc = tc.nc.named_scope("A_bqk"); sc.__enter__()
                psb_pool = tc.tile_pool(name="psb", bufs=1, space="PSUM")
                ppb = psb_pool.__enter__()
                ps_b = ppb.tile([P, NKV], FP, tag="psb")
                nop = 0
                for kvt in range(NKV):
                    ksl = slice(kvt * P, (kvt + 1) * P)
                    for t in range(NT // 2):
                        nc.tensor.matmul(
                            ps_b[:, kvt : kvt + 1],
                            lhsT=kT[:, 1, 2 * t : 2 * t + 2, ksl],
                            rhs=bqp[:, 0, 2 * t : 2 * t + 2, :],
                            start=(nop == 0),
                            stop=False,
                            perf_mode=DR,
                            skip_group_check=True,
                        )
                        nop += 1
                    for ct in range(NT):
                        nop += 1
                        nc.tensor.matmul(
                            ps_b[:, kvt : kvt + 1],
                            lhsT=kT[:, 0:2, ct, ksl],
                            rhs=bqp[:, 0:2, ct, :],
                            start=False,
                            stop=(nop == 12 * NKV),
                            perf_mode=DR,
                            skip_group_check=True,
                        )
                # bqk_sb = bqk - ln(ASCALE): the Exp bias for each kv row
                nc.scalar.activation(
                    bqk_sb[:],
                    ps_b[:],
                    AF.Identity,
                    scale=1.0 / 1048576.0,
                    bias=bias_sb[:, NT : NT + 1],
                )
                # boosted variant (no -ln32): exp stored unscaled for the
                # earliest query columns, whose tiny softmax supports would
                # otherwise sink into the fp8 subnormal floor. A per-column
                # exp scale cancels between att@V and the rowsum.
                nc.scalar.activation(
                    bqk_sb2[:],
                    ps_b[:],
                    AF.Identity,
                    scale=1.0 / 1048576.0,
                    bias=bias_sb[:, NT + 2 : NT + 3],
                )
                psb_pool.__exit__(None, None, None)
                sc.__exit__(None, None, None)

                # ---- v projection (x stationary, w moving; no bias) -----
                sc = tc.nc.named_scope("A_v"); sc.__enter__()
                for half in range(2):
                    for tt in range(NT):
                        ts_g = half * NT + tt
                        xh, tb = vtile(half, tt)
                        tsl = slice(tb, tb + P)
                        for cs, ce in ((0, 512), (512, 1024)):
                            ps = pp.tile([P, 512], FP, tag="ps")
                            first = True
                            for t in range(NT // 2):
                                nc.tensor.matmul(
                                    ps[:],
                                    lhsT=xh[:, 0, 2 * t : 2 * t + 2, tsl],
                                    rhs=wv[:, 1, 2 * t : 2 * t + 2, cs:ce],
                                    start=first,
                                    stop=False,
                                    perf_mode=DR,
                                )
                                first = False
                            for ct in range(NT):
                                nc.tensor.matmul(
                                    ps[:],
                                    lhsT=xh[:, 0:2, ct, tsl],
                                    rhs=wv[:, 0:2, ct, cs:ce],
                                    start=False,
                                    stop=(ct == NT - 1),
                                    perf_mode=DR,
                                )
                            nc.scalar.activation(
                                V[:, 1, ts_g, cs:ce], ps[:], AF.Identity
                            )
                            nc.vector.tensor_sub(
                                V[:, 0, ts_g, cs:ce], ps[:], V[:, 1, ts_g, cs:ce]
                            )
                sc.__exit__(None, None, None)

                # ---- q projection (scaled wq; bias via bqk) -------------
                sc = tc.nc.named_scope("A_q"); sc.__enter__()
                for ot in range(NT):
                    osl = slice(ot * P, (ot + 1) * P)
                    for cs, ce in ((0, 512), (512, 1024)):
                        ps = pp.tile([P, 512], FP, tag="ps")
                        mm12(ps, wq, xq, osl, cs, ce)
                        nc.scalar.activation(
                            qT[:, 0, ot, cs:ce], ps[:], AF.Identity
                        )
                        nc.vector.tensor_sub(
                            qT[:, 1, ot, cs:ce], ps[:], qT[:, 0, ot, cs:ce]
                        )
                sc.__exit__(None, None, None)

            # =============================================================
            # Phases B-D (attention): xw freed; AT/Oacc/Opair reuse space
            # =============================================================
            with (
                tc.tile_pool(name="attn", bufs=1) as ab,
                tc.tile_pool(name="efp", bufs=5) as efp,
                tc.tile_pool(name="psum_rs", bufs=1, space="PSUM") as pp_rs,
            ):
                # per-pair AT tiles and per-column-half O tiles: tile deps
                # are whole-tile, so consumers must not share tiles with
                # later producers
                ATp = [
                    ab.tile([P, 2, 2, H], F8, tag=f"ATp{p}", name=f"ATp{p}")
                    for p in range(NKV // 2)
                ]  # [comp(hi,lo), sub-in-pair, qcol]
                Oaccs = [
                    ab.tile([P, NT, 512], FP, tag=f"Oacc{ci}", name=f"Oacc{ci}")
                    for ci in range(2)
                ]
                Ops = [
                    ab.tile([P, 2, NT, 512], F8, tag=f"Op{ci}", name=f"Op{ci}")
                    for ci in range(2)
                ]  # (hi, lo)
                rsbs = [
                    ab.tile([P, 512], FP, tag=f"rsb{ci}", name=f"rsb{ci}")
                    for ci in range(2)
                ]
                rs_ps = pp_rs.tile([P, H], FP, tag="rs")

                ef_cur = [None]
                pend = [None]  # delayed pair decompose closure

                def scores_s(s):
                    lo_s = LO16[s] * P
                    lo_p = LOP16[s] * P
                    if s % 2 == 0:
                        ef_cur[0] = efp.tile([P, 2, H], BF, tag="ef", name=f"ef{s}")
                    ef = ef_cur[0]
                    chs = _chunks512(lo_p, H)
                    pss = [
                        pp.tile([P, ce - cs], FP, tag="ps", name=f"pss{s}_{cs}")
                        for cs, ce in chs
                    ]
                    # ct-outer so each stationary kT slice loads once
                    nop = 0
                    for t in range(NT // 2):
                        for ps, (cs, ce) in zip(pss, chs):
                            mlo = max(cs, lo_s)
                            nc.tensor.matmul(
                                ps[:, mlo - cs : ce - cs],
                                lhsT=kT[:, 1, 2 * t : 2 * t + 2, s * P : (s + 1) * P],
                                rhs=qT[:, 0, 2 * t : 2 * t + 2, mlo:ce],
                                start=(nop < len(chs)),
                                stop=False,
                                perf_mode=DR,
                                skip_group_check=True,
                            )
                            nop += 1
                    for ct in range(NT):
                        for ps, (cs, ce) in zip(pss, chs):
                            mlo = max(cs, lo_s)
                            nc.tensor.matmul(
                                ps[:, mlo - cs : ce - cs],
                                lhsT=kT[:, 0:2, ct, s * P : (s + 1) * P],
                                rhs=qT[:, 0:2, ct, mlo:ce],
                                start=False,
                                stop=(ct == NT - 1),
                                perf_mode=DR,
                                skip_group_check=True,
                            )
                    for ps, (cs, ce) in zip(pss, chs):
                        for blk, mi in MASKS16[s]:
                            a = blk * P
                            if cs <= a < ce:
                                nc.vector.tensor_add(
                                    ps[:, a - cs : a - cs + P],
                                    ps[:, a - cs : a - cs + P],
                                    masks[:, mi, :],
                                )
                    if pend[0] is not None:
                        pend[0]()
                        pend[0] = None
                    # dead sliver [lo_p, lo_s): set to -1e9 on Act (scale=0
                    # kills the garbage psum) so exp = 0 there and the fp8
                    # pair reads as exact zeros for the paired attv ops
                    if lo_s > lo_p:
                        nc.scalar.activation(
                            pss[0][:, 0 : lo_s - lo_p],
                            pss[0][:, 0 : lo_s - lo_p],
                            AF.Identity,
                            bias=bias_sb[:, NT + 1 : NT + 2],
                            scale=0.0,
                        )
                    for ps, (cs, ce) in zip(pss, chs):
                        segs = (
                            [(cs, P, bqk_sb2), (P, ce, bqk_sb)]
                            if (s < 4 and cs == 0)
                            else [(cs, ce, bqk_sb)]
                        )
                        for g0, g1, bq_t in segs:
                            nc.scalar.activation(
                                ef[:, s % 2, g0:g1],
                                ps[:, g0 - cs : g1 - cs],
                                AF.Exp,
                                scale=1.0 / 32768.0,
                                bias=bq_t[:, s : s + 1],
                            )
                    if s % 2 == 1:
                        pend[0] = lambda s=s, ef=ef, lo_p=lo_p: _decompose(s, ef, lo_p)

                def _decompose(s, ef, lo_p):
                    if True:
                        # pair complete: decompose exp into the AT fp8 pair
                        # (hi copy on Pool, lo sub on DVE -- splits the ~28us
                        # of decompose across the two engines with slack)
                        atp = ATp[s // 2]
                        nc.gpsimd.tensor_copy(
                            atp[:, 0, :, lo_p:H], ef[:, :, lo_p:H]
                        )
                        if lo_p < 512:
                            nc.gpsimd.tensor_sub(
                                atp[:, 1, :, lo_p:512],
                                ef[:, :, lo_p:512],
                                atp[:, 0, :, lo_p:512],
                            )
                        nc.vector.tensor_sub(
                            atp[:, 1, :, 512:H],
                            ef[:, :, 512:H],
                            atp[:, 0, :, 512:H],
                        )
                        # rowsum for both subtiles of the pair
                        for sj in (s - 1, s):
                            lo_j = LOP16[sj] * P
                            for cs, ce in _chunks512(lo_j, H):
                                nc.tensor.matmul(
                                    rs_ps[:, cs:ce],
                                    lhsT=ones_sb[:],
                                    rhs=atp[:, 0:2, sj % 2, cs:ce],
                                    start=(sj == 0),
                                    stop=(
                                        (cs < 512 and sj == 7)
                                        or (cs >= 512 and sj == NKV - 1)
                                    ),
                                    perf_mode=DR,
                                    skip_group_check=True,
                                )

                def attv_chunk(ci, pairs):
                    cs, ce = ci * 512, (ci + 1) * 512
                    for ot in range(NT):
                        osl = slice(ot * P, (ot + 1) * P)
                        ps = pp.tile([P, 512], FP, tag="ps")
                        nops = 3 * len(pairs)
                        nop = 0
                        for p in pairs:
                            plo = max(cs, LOP16[2 * p] * P)
                            nc.tensor.matmul(
                                ps[:, plo - cs : 512],
                                lhsT=V[:, 1, 2 * p : 2 * p + 2, osl],
                                rhs=ATp[p][:, 0, :, plo:ce],
                                start=(nop == 0),
                                stop=False,
                                perf_mode=DR,
                                skip_group_check=True,
                            )
                            nop += 1
                            for sj in (0, 1):
                                nop += 1
                                nc.tensor.matmul(
                                    ps[:, plo - cs : 512],
                                    lhsT=V[:, 0:2, 2 * p + sj, osl],
                                    rhs=ATp[p][:, 0:2, sj, plo:ce],
                                    start=False,
                                    stop=(nop == nops),
                                    perf_mode=DR,
                                    skip_group_check=True,
                                )
                        nc.vector.tensor_copy(Oaccs[ci][:, ot, :], ps[:])

                def normalize_chunk(ci):
                    cs, ce = ci * 512, (ci + 1) * 512
                    Oc, Op, rsb = Oaccs[ci], Ops[ci], rsbs[ci]
                    nc.vector.tensor_copy(rsb[:], rs_ps[:, cs:ce])
                    nc.vector.reciprocal(rsb[:], rsb[:])
                    for ot in range(NT):
                        nc.vector.tensor_mul(Oc[:, ot, :], Oc[:, ot, :], rsb[:])
                        nc.scalar.activation(
                            Op[:, 0, ot, :], Oc[:, ot, :], AF.Identity
                        )
                        nc.vector.tensor_sub(
                            Op[:, 1, ot, :], Oc[:, ot, :], Op[:, 0, ot, :]
                        )

                def oproj_chunk(ci, evac):
                    Op = Ops[ci]
                    for o2 in range(NT):
                        osl = slice(o2 * P, (o2 + 1) * P)
                        ps = pp.tile([P, 512], FP, tag="ps", name=f"pso{ci}_{o2}")
                        first = True
                        for t in range(NT // 2):
                            nc.tensor.matmul(
                                ps[:],
                                lhsT=wp[:, 1, 2 * t : 2 * t + 2, osl],
                                rhs=Op[:, 0, 2 * t : 2 * t + 2, :],
                                start=first,
                                stop=False,
                                perf_mode=DR,
                            )
                            first = False
                        for ct in range(NT):
                            nc.tensor.matmul(
                                ps[:],
                                lhsT=wp[:, 0:2, ct, osl],
                                rhs=Op[:, 0:2, ct, :],
                                start=False,
                                stop=(ct == NT - 1),
                                perf_mode=DR,
                            )
                        ev = evac.tile([P, 512], BF, tag="evy", name=f"evy{ci}_{o2}")
                        nc.scalar.activation(
                            ev[:],
                            ps[:],
                            AF.Identity,
                            scale=1.0 / 1024.0,
                            bias=bias_sb[:, o2 : o2 + 1],
                        )
                        nc.sync.dma_start(
                            yT[(o2 * 2 + ci) * P : (o2 * 2 + ci + 1) * P, :],
                            ev[:],
                        )

                sc = tc.nc.named_scope("B1"); sc.__enter__()
                for s in range(16):
                    scores_s(s)
                if pend[0] is not None:
                    pend[0]()
                    pend[0] = None
                sc.__exit__(None, None, None)
                with tc.tile_pool(name="evac", bufs=3) as evac:
                    sc = tc.nc.named_scope("B2"); sc.__enter__()
                    attv_chunk(0, [0, 1, 2, 3])
                    normalize_chunk(0)
                    sc.__exit__(None, None, None)
                    sc = tc.nc.named_scope("D2"); sc.__enter__()
                    attv_chunk(1, [0, 1, 2, 3, 4, 5, 6, 7])
                    normalize_chunk(1)
                    sc.__exit__(None, None, None)
                    sc = tc.nc.named_scope("E"); sc.__enter__()
                    oproj_chunk(0, evac)
                    oproj_chunk(1, evac)
                    sc.__exit__(None, None, None)

    _split_waits(nc)
    return nc


_NC_CACHE = None


def _get_nc():
    global _NC_CACHE
    if _NC_CACHE is None:
        _NC_CACHE = _build_nc()
    return _NC_CACHE


def _pair(a, order="hl"):
    """Decompose fp32 array -> fp8 (hi, lo) or (lo, hi) pair along new axis 1.
    a: [P, ...]; returns [P, 2, ...] float8_e4m3."""
    import ml_dtypes

    a = np.asarray(a, dtype=np.float32)
    hi = a.astype(ml_dtypes.float8_e4m3)
    lo = (a - hi.astype(np.float32)).astype(ml_dtypes.float8_e4m3)
    pair = (hi, lo) if order == "hl" else (lo, hi)
    return np.ascontiguousarray(np.stack(pair, axis=1))


def _tile_major(m):
    """[C_in, N] -> [P, C_in//P, N] with partition dim first."""
    cin, n = m.shape
    return np.ascontiguousarray(m.reshape(cin // P, P, n).transpose(1, 0, 2))


def make_in_maps(x, w_qkv, b_qkv, w_proj, b_proj):
    """Host-side prep: shard + transpose + fp8-decompose inputs for 8 cores."""
    x = np.asarray(x, dtype=np.float32)
    w_qkv = np.asarray(w_qkv, dtype=np.float32)
    b_qkv = np.asarray(b_qkv, dtype=np.float32)
    w_proj = np.asarray(w_proj, dtype=np.float32)
    b_proj = np.asarray(b_proj, dtype=np.float32)
    import ml_dtypes

    s = 1.0 / np.sqrt(np.float32(C))

    # weights x32 so every fp8 operand sits at sigma >= 1 (far above the
    # e4m3 subnormal floor); the excess scale folds into Act scale args.
    # stored (lo, hi), layout [P cpart, 2, NT csub, C out]
    wq = _pair(_tile_major(w_qkv[0:C].T * 32.0), "lh")
    wk = _pair(_tile_major(w_qkv[C : 2 * C].T * 32.0), "lh")
    # per-ot contiguous blocks so each wk DMA has 2KB descriptor runs
    wk = np.ascontiguousarray(
        np.moveaxis(wk.reshape(P, 2, NT, NT, P), 3, 0)
    )
    wv = _pair(_tile_major(w_qkv[2 * C : 3 * C].T * 32.0), "lh")
    wp = _pair(_tile_major(w_proj.T * 32.0), "lh")
    bqp = _pair(_tile_major(b_qkv[0:C].reshape(C, 1) * 1024.0), "hl")

    bv = b_qkv[2 * C : 3 * C]
    beff = (b_proj + w_proj @ bv).reshape(NT, P).T
    bias = np.concatenate(
        [
            beff,
            np.full((P, 1), -np.log(ASCALE), np.float32),
            np.full((P, 1), NEG, np.float32),
            np.full((P, 1), -np.log(4.0), np.float32),
        ],
        axis=1,
    ).astype(np.float32)

    ones = np.ones((P, 2, P), dtype=np.float32).astype(ml_dtypes.float8_e4m3)

    # S^T mask tiles: partition = kv j (within subtile), free = query i
    triu = np.triu(np.ones((P, P), dtype=np.float32))
    trilm = np.where(triu > 0, 0.0, NEG).astype(np.float32)
    zeros = np.zeros((P, P), dtype=np.float32)
    negs = np.full((P, P), NEG, dtype=np.float32)

    shared = dict(
        wq_in=wq, wk_in=wk, wv_in=wv, wp_in=wp, bqp_in=bqp,
        bias_in=bias, ones_in=ones,
    )
    in_maps = []
    for core in range(8):
        b, h = core // 2, core % 2
        xb = x[b]  # [T, C]
        qrows = np.concatenate(
            [xb[(2 * bg + h) * 256 : (2 * bg + h + 1) * 256] for bg in range(4)],
            axis=0,
        )
        xo_pair = _pair(_tile_major(xb[0:H].T), "hl")
        m = np.stack(
            [
                trilm if h == 0 else zeros,   # m1d
                negs if h == 0 else zeros,    # m1f
                negs if h == 0 else trilm,    # m2d
            ],
            axis=1,
        )
        in_maps.append(
            dict(
                shared,
                xq_in=_pair(_tile_major(qrows.T), "hl"),
                xoA_in=np.ascontiguousarray(xo_pair[:, :, :, 0:256]),
                xoB_in=np.ascontiguousarray(xo_pair[:, :, :, 256:512]),
                xoC_in=np.ascontiguousarray(xo_pair[:, :, :, 512:1024]),
                xx_in=_pair(_tile_major(xb[H : 2 * H].T), "hl"),
                masks_in=np.ascontiguousarray(m),
            )
        )
    return in_maps


def assemble_output(results):
    B = 4
    y = np.empty((B, 2 * H, C), dtype=np.float32)
    for core in range(8):
        b, h = core // 2, core % 2
        yt = results[core]["yT"].astype(np.float32).reshape(NT, 2, P, 512)
        blk = yt.transpose(1, 3, 0, 2).reshape(H, C)
        blk4 = blk.reshape(4, 256, C)
        for bg in range(4):
            g = 2 * bg + h
            y[b, g * 256 : (g + 1) * 256, :] = blk4[bg]
    return y


def kernel(x, w_qkv, b_qkv, w_proj, b_proj):
    from concourse.bass_utils import run_bass_kernel_spmd

    nc = _get_nc()
    in_maps = make_in_maps(x, w_qkv, b_qkv, w_proj, b_proj)
    res = run_bass_kernel_spmd(nc, in_maps, list(range(8)))
    return assemble_output(res.results)


# revision 34
# speedup vs baseline: 1.0170x; 1.0170x over previous
"""Single-head causal attention (B=4, T=2048, C=1024) on 8 trn2 NeuronCores.

Sharding: 8 shards = (batch b in 0..3) x (query interleave h in 0..1), same
balanced interleaved-256 query split as the fp32r baseline: core h of a pair
takes global 256-row query blocks {2*bg+h}, so the causal triangle is
balanced across the pair. One SPMD instruction stream; all per-core
variation is data (gathered x slices + three [128,128] mask tiles).

All matmuls run as compensated fp8e4m3 DoubleRow pairs. Each operand is
decomposed as v = hi + lo (hi = fp8(v), lo = fp8(v - hi)); a product
x*w = xh*wh + (xh*wl + xl*wh) keeps ~bf16 accuracy (residual ~0.1%) while
DoubleRow processes TWO 128-deep contraction planes per instruction at 0.5
cycles/row -- 4x the fp32r/bf16 rate, so the compensated triple costs 0.75x
of the bf16-equivalent. Host-side tensors (x, weights) are decomposed on
the host; device-computed tensors (q, k, v, exp-scores, attention out) are
decomposed with a cast + subtract pass (Act/DVE/Pool engines, all far under
the PE roofline).

Algebraic folds:
  - k bias dropped entirely: softmax over kv positions is invariant to
    per-query constants, and (q+bq).(k+bk) - (q+bq).k is constant per row.
  - q bias folded into the exp: s_ij = q~_i.k_j + (bq~.k_j), the second
    term is per-kv-position, computed on device as a tiny N=1 DoubleRow
    matmul chain (bqk), and applied as the Exp activation bias together
    with -ln(32).
  - exp(s)/32 stored instead of exp(s) so fp8's 240 max is never hit
    (scores ~N(0,1); the 1/32 cancels between att@V and the rowsum).
  - v bias folded into the output bias (beff = b_proj + w_proj @ b_v).
  - 1/sqrt(C) folded into wq/bq host-side.

fp8 halves SBUF, so unlike the baseline there is NO DRAM spill of kv half 1
(phases C/skT/sV are gone): kT/V for all 2048 kv positions, qT, AT and the
fp32 O accumulator are all resident.

Comp-plane storage convention (so compensated cross terms pair cleanly):
"moving-side" tensors (x, qT, AT, Opair, bq) store (hi, lo); "stationary
side" (wk, wq, wv, wp, kT, V) store (lo, hi). A cross op then reads
lhsT[:, 0:2] x rhs[:, 0:2] = wl*xh + wh*xl directly; hi*hi ops index plane
1 of the stationary and plane 0 of the moving tensor, pairing adjacent
contraction subtiles instead.
"""

import sys

sys.path.insert(0, "/opt/trn_rl_repo")

import numpy as np

import concourse.bass as bass
import concourse.tile as tile
from concourse import mybir
from concourse.vector_clock import ScopedClock

FP = mybir.dt.float32
BF = mybir.dt.bfloat16
F8 = mybir.dt.float8e4
AF = mybir.ActivationFunctionType
DR = mybir.MatmulPerfMode.DoubleRow

P = 128
C = 1024  # embed dim
H = 1024  # query rows per core
TKV = 2048  # kv length
NT = C // P  # 8 c-subtiles
NKV = TKV // P  # 16 kv-subtiles
NEG = -1.0e9
ASCALE = 32.0  # exp(s)/ASCALE stored in fp8

_MAX_WAITS = 1


class _TC(tile.TileContext):
    """TileContext whose tail drain puts its global-clock waits on a nop
    (walrus rejects multi-wait Drain); excess waits are split by
    _split_waits() afterwards."""

    def _drain_and_barrier(self, tick_clock, wait_clock):
        nop_inst = self.nc.sync.nop(nofuse=True, hint="pre_drain_waits")
        wait_clock.add_sem_waits(
            nop_inst.ins, ScopedClock({None: tick_clock.global_clock})
        )
        self.nc.sync.drain()
        self.nc.all_engine_barrier()
        assert self.sems is not None
        popped = self.nc._tile_sem_poison_stack.pop()
        assert popped is self._sem_poison
        self.nc.clear_and_free_semaphores(list(self.sems.allocated().values()))
        self.nc.all_engine_barrier()


def _split_waits(nc, max_waits=_MAX_WAITS):
    """The walrus shipped here rejects instructions carrying more than
    `max_waits` sync waits. Move excess waits onto injected nops placed
    immediately before the instruction on the same engine."""
    import copy

    template = nc.sync.nop(nofuse=True, hint="waitsplit_template").ins
    counter = [0]

    def make_nop(engine, waits):
        nop = copy.deepcopy(template)
        counter[0] += 1
        nop.name = f"I-wsplit-{counter[0]}"
        nop.engine = engine
        nop.sync_info = mybir.SyncInfo(on_wait=list(waits), on_update=[])
        return nop

    f = nc.m.functions[0]
    for bb in f.blocks:
        insts = bb.instructions
        if not any(
            i.sync_info and i.sync_info.on_wait and len(i.sync_info.on_wait) > max_waits
            for i in insts
        ):
            continue
        newlist = []
        for inst in insts:
            si = inst.sync_info
            if si and si.on_wait and len(si.on_wait) > max_waits:
                if inst.name == template.name:
                    newlist.append(inst)
                    continue
                waits = list(si.on_wait)
                del si.on_wait[max_waits:]
                rest = waits[max_waits:]
                while rest:
                    newlist.append(make_nop(inst.engine, rest[:max_waits]))
                    rest = rest[max_waits:]
            newlist.append(inst)
        bb.instructions[:] = newlist


# Causal structure for the interleaved-256 query sharding, over 16 kv
# 128-subtiles. Query slots bg=0..3 hold global 256-row blocks g=2*bg+h.
# For kv subtile s, valid query cols start at LO16[s]*128; mask tiles
# (data-encoded per core) are added at the listed 128-col block positions.
LO16 = [0, 0, 0, 1, 2, 2, 2, 3, 4, 4, 4, 5, 6, 6, 6, 7]
_MASKS8 = [
    [(0, 0)],            # (128-block, mask index) ; 0=m1d 1=m1f 2=m2d
    [(0, 1), (1, 0)],
    [(0, 2), (1, 1)],
    [(1, 2)],
    [(2, 0)],
    [(2, 1), (3, 0)],
    [(2, 2), (3, 1)],
    [(3, 2)],
]
MASKS16 = [
    [((s // 8) * 4 + off, mi) for off, mi in _MASKS8[s % 8]] for s in range(16)
]
# pair-aligned lo (attv pairs kv subtiles (2p, 2p+1))
LOP16 = [LO16[s] - (LO16[s] % 2) for s in range(16)]


def _chunks512(lo, hi):
    """Split [lo, hi) at absolute multiples of 512."""
    out = []
    while lo < hi:
        ce = min((lo // 512 + 1) * 512, hi)
        out.append((lo, ce))
        lo = ce
    return out


def _build_nc():
    nc = bass.Bass("TRN2", target_bir_lowering=False, debug=False)

    xq_in = nc.dram_tensor("xq_in", [P, 2, NT, H], F8, kind="ExternalInput").ap()
    xoA_in = nc.dram_tensor("xoA_in", [P, 2, NT, 256], F8, kind="ExternalInput").ap()
    xoB_in = nc.dram_tensor("xoB_in", [P, 2, NT, 256], F8, kind="ExternalInput").ap()
    xoC_in = nc.dram_tensor("xoC_in", [P, 2, NT, 512], F8, kind="ExternalInput").ap()
    xx_in = nc.dram_tensor("xx_in", [P, 2, NT, H], F8, kind="ExternalInput").ap()
    wk_in = nc.dram_tensor("wk_in", [NT, P, 2, NT, P], F8, kind="ExternalInput").ap()
    wq_in = nc.dram_tensor("wq_in", [P, 2, NT, C], F8, kind="ExternalInput").ap()
    wv_in = nc.dram_tensor("wv_in", [P, 2, NT, C], F8, kind="ExternalInput").ap()
    wp_in = nc.dram_tensor("wp_in", [P, 2, NT, C], F8, kind="ExternalInput").ap()
    bqp_in = nc.dram_tensor("bqp_in", [P, 2, NT, 1], F8, kind="ExternalInput").ap()
    ones_in = nc.dram_tensor("ones_in", [P, 2, P], F8, kind="ExternalInput").ap()
    masks_in = nc.dram_tensor("masks_in", [P, 3, P], FP, kind="ExternalInput").ap()
    # beff (8 cols) | -ln(ASCALE) | -1e9 sliver-kill | 0 (boosted-exp bias)
    bias_in = nc.dram_tensor("bias_in", [P, NT + 3], FP, kind="ExternalInput").ap()
    # output, (o2-tile, chunk)-major, bf16; host reassembles + upcasts
    yT = nc.dram_tensor("yT", [NT * 2 * P, 512], BF, kind="ExternalOutput").ap()

    with _TC(nc) as tc:
        with (
            tc.tile_pool(name="misc", bufs=1) as misc,
            tc.tile_pool(name="kqv", bufs=1) as kqv,
            tc.tile_pool(name="psum", bufs=6, space="PSUM") as pp,
        ):
            ones_sb = misc.tile([P, 2, P], F8, tag="ones")
            masks = misc.tile([P, 3, P], FP, tag="masks")
            bias_sb = misc.tile([P, NT + 3], FP, tag="bias")
            bqp = misc.tile([P, 2, NT, 1], F8, tag="bqp")
            bqk_sb = misc.tile([P, NKV], FP, tag="bqk")
            bqk_sb2 = misc.tile([P, NKV], FP, tag="bqk2")

            # persistent fp8 pair tensors (comp order noted)
            kT = kqv.tile([P, 2, NT, TKV], F8, tag="kT")   # (lo, hi)
            qT = kqv.tile([P, 2, NT, H], F8, tag="qT")     # (hi, lo)
            V = kqv.tile([P, 2, NKV, C], F8, tag="V")      # (lo, hi)
            wp = kqv.tile([P, 2, NT, C], F8, tag="wp")     # (lo, hi)

            # =============================================================
            # Phase A: projections
            # =============================================================
            with tc.tile_pool(name="xw", bufs=1) as xw:
                # x half-0 split into 3 tiles, wk into 8 per-ot tiles:
                # tile deps are per-tile, so fine-grained tiles let the first
                # kproj group start as soon as its own DMAs land
                xoA = xw.tile([P, 2, NT, 256], F8, tag="xoA")
                xoB = xw.tile([P, 2, NT, 256], F8, tag="xoB")
                xoC = xw.tile([P, 2, NT, 512], F8, tag="xoC")
                xx = xw.tile([P, 2, NT, H], F8, tag="xx")
                xq = xw.tile([P, 2, NT, H], F8, tag="xq")
                wk = [
                    xw.tile([P, 2, NT, P], F8, tag=f"wk{ot}", name=f"wk{ot}")
                    for ot in range(NT)
                ]
                wq = xw.tile([P, 2, NT, C], F8, tag="wq")
                wv = xw.tile([P, 2, NT, C], F8, tag="wv")

                # fine-grained first loads so kproj starts ASAP
                nc.sync.dma_start(wk[0][:], wk_in[0])
                nc.sync.dma_start(xoA[:], xoA_in[:])
                for ot in range(1, NT):
                    nc.sync.dma_start(wk[ot][:], wk_in[ot])
                nc.sync.dma_start(xoB[:], xoB_in[:])
                nc.sync.dma_start(xoC[:], xoC_in[:])
                nc.sync.dma_start(xx[:], xx_in[:])
                nc.sync.dma_start(ones_sb[:], ones_in[:])
                nc.sync.dma_start(masks[:], masks_in[:])
                nc.sync.dma_start(bias_sb[:], bias_in[:])
                nc.sync.dma_start(bqp[:], bqp_in[:])
                nc.sync.dma_start(xq[:], xq_in[:])
                nc.sync.dma_start(wq[:], wq_in[:])
                nc.sync.dma_start(wv[:], wv_in[:])
                nc.sync.dma_start(wp[:], wp_in[:])

                # PE p-state warmup on a memset tile: no DMA dependency, so
                # the ramp completes while the first x/w transfers land
                wsrc = xw.tile([P, 2, P], F8, tag="wsrc")
                nc.vector.memset(wsrc[:], 1.0)
                wps = pp.tile([P, 512], FP, tag="ps", name="wps")
                for _ in range(40):
                    nc.tensor.matmul(
                        wps[:, 0:P],
                        lhsT=wsrc[:],
                        rhs=wsrc[:, :, :],
                        start=True,
                        stop=True,
                        perf_mode=DR,
                        skip_group_check=True,
                    )

                # (x tile, local col range, global kv base) pieces
                kchunks = [
                    (xoA, 0, 256, 0),
                    (xoB, 0, 256, 256),
                    (xoC, 0, 512, 512),
                    (xx, 0, 512, 1024),
                    (xx, 512, 1024, 1536),
                ]
                # vproj: token-tile tt of half -> (x tile, local col base)
                def vtile(half, tt):
                    if half == 1:
                        return xx, tt * P
                    if tt < 2:
                        return xoA, tt * P
                    if tt < 4:
                        return xoB, (tt - 2) * P
                    return xoC, (tt - 4) * P

                def mm12(ps, w, x, osl, cs, ce, n_start=True, n_stop=True):
                    """12-op compensated group: out[osl, cs:ce] += w.T @ x.
                    w stored (lo,hi), x stored (hi,lo); contraction over all
                    NT c-subtiles."""
                    first = [n_start]
                    for t in range(NT // 2):
                        nc.tensor.matmul(
                            ps[:, : ce - cs],
                            lhsT=w[:, 1, 2 * t : 2 * t + 2, osl],
                            rhs=x[:, 0, 2 * t : 2 * t + 2, cs:ce],
                            start=first[0],
                            stop=False,
                            perf_mode=DR,
                        )
                        first[0] = False
                    for ct in range(NT):
                        nc.tensor.matmul(
                            ps[:, : ce - cs],
                            lhsT=w[:, 0:2, ct, osl],
                            rhs=x[:, 0:2, ct, cs:ce],
                            start=False,
                            stop=(n_stop and ct == NT - 1),
                            perf_mode=DR,
                        )

                # ---- k projection (no bias; softmax-invariant) ----------
                sc = tc.nc.named_scope("A_k"); sc.__enter__()
                for xh, cs, ce, gb in kchunks:
                    for ot in range(NT):
                        ps = pp.tile([P, 512], FP, tag="ps", name=f"psk{gb}_{ot}")
                        mm12(ps, wk[ot], xh, slice(0, P), cs, ce)
                        g0, g1 = gb, gb + (ce - cs)
                        nc.scalar.activation(
                            kT[:, 1, ot, g0:g1], ps[:, : ce - cs], AF.Identity
                        )
                        nc.vector.tensor_sub(
                            kT[:, 0, ot, g0:g1],
                            ps[:, : ce - cs],
                            kT[:, 1, ot, g0:g1],
                        )
                sc.__exit__(None, None, None)

                # ---- bqk: per-kv-position q-bias term (bq~ . k_j) -------
                sc = tc.nc.named_scope("A_bqk"); sc.__enter__()
                psb_pool = tc.tile_pool(name="psb", bufs=1, space="PSUM")
                ppb = psb_pool.__enter__()
                ps_b = ppb.tile([P, NKV], FP, tag="psb")
                nop = 0
                for kvt in range(NKV):
                    ksl = slice(kvt * P, (kvt + 1) * P)
                    for t in range(NT // 2):
                        nc.tensor.matmul(
                            ps_b[:, kvt : kvt + 1],
                            lhsT=kT[:, 1, 2 * t : 2 * t + 2, ksl],
                            rhs=bqp[:, 0, 2 * t : 2 * t + 2, :],
                            start=(nop == 0),
                            stop=False,
                            perf_mode=DR,
                            skip_group_check=True,
                        )
                        nop += 1
                    for ct in range(NT):
                        nop += 1
                        nc.tensor.matmul(
                            ps_b[:, kvt : kvt + 1],
                            lhsT=kT[:, 0:2, ct, ksl],
                            rhs=bqp[:, 0:2, ct, :],
                            start=False,
                            stop=(nop == 12 * NKV),
                            perf_mode=DR,
                            skip_group_check=True,
                        )
                # bqk_sb = bqk - ln(ASCALE): the Exp bias for each kv row
                nc.scalar.activation(
                    bqk_sb[:],
                    ps_b[:],
                    AF.Identity,
                    scale=1.0 / 1048576.0,
                    bias=bias_sb[:, NT : NT + 1],
                )
                # boosted variant (no -ln32): exp stored unscaled for the
                # earliest query columns, whose tiny softmax supports would
                # otherwise sink into the fp8 subnormal floor. A per-column
                # exp scale cancels between att@V and the rowsum.
                nc.scalar.activation(
                    bqk_sb2[:],
                    ps_b[:],
                    AF.Identity,
                    scale=1.0 / 1048576.0,
                    bias=bias_sb[:, NT + 2 : NT + 3],
                )
                psb_pool.__exit__(None, None, None)
                sc.__exit__(None, None, None)

                # ---- v projection (x stationary, w moving; no bias) -----
                sc = tc.nc.named_scope("A_v"); sc.__enter__()
                for half in range(2):
                    for tt in range(NT):
                        ts_g = half * NT + tt
                        xh, tb = vtile(half, tt)
                        tsl = slice(tb, tb + P)
                        for cs, ce in ((0, 512), (512, 1024)):
                            ps = pp.tile([P, 512], FP, tag="ps")
                            first = True
                            for t in range(NT // 2):
                                nc.tensor.matmul(
                                    ps[:],
                                    lhsT=xh[:, 0, 2 * t : 2 * t + 2, tsl],
                                    rhs=wv[:, 1, 2 * t : 2 * t + 2, cs:ce],
                                    start=first,
                                    stop=False,
                                    perf_mode=DR,
                                )
                                first = False
                            for ct in range(NT):
                                nc.tensor.matmul(
                                    ps[:],
                                    lhsT=xh[:, 0:2, ct, tsl],
                                    rhs=wv[:, 0:2, ct, cs:ce],
                                    start=False,
                                    stop=(ct == NT - 1),
                                    perf_mode=DR,
                                )
                            nc.scalar.activation(
                                V[:, 1, ts_g, cs:ce], ps[:], AF.Identity
                            )
                            nc.vector.tensor_sub(
                                V[:, 0, ts_g, cs:ce], ps[:], V[:, 1, ts_g, cs:ce]
                            )
                sc.__exit__(None, None, None)

                # ---- q projection (scaled wq; bias via bqk) -------------
                sc = tc.nc.named_scope("A_q"); sc.__enter__()
                for ot in range(NT):
                    osl = slice(ot * P, (ot + 1) * P)
                    for cs, ce in ((0, 512), (512, 1024)):
                        ps = pp.tile([P, 512], FP, tag="ps")
                        mm12(ps, wq, xq, osl, cs, ce)
                        nc.scalar.activation(
                            qT[:, 0, ot, cs:ce], ps[:], AF.Identity
                        )
                        nc.vector.tensor_sub(
                            qT[:, 1, ot, cs:ce], ps[:], qT[:, 0, ot, cs:ce]
                        )
                sc.__exit__(None, None, None)

            # =============================================================
            # Phases B-D (attention): xw freed; AT/Oacc/Opair reuse space
            # =============================================================
            with (
                tc.tile_pool(name="attn", bufs=1) as ab,
                tc.tile_pool(name="efp", bufs=5) as efp,
                tc.tile_pool(name="psum_rs", bufs=1, space="PSUM") as pp_rs,
            ):
                # per-pair AT tiles and per-column-half O tiles: tile deps
                # are whole-tile, so consumers must not share tiles with
                # later producers
                ATp = [
                    ab.tile([P, 2, 2, H], F8, tag=f"ATp{p}", name=f"ATp{p}")
                    for p in range(NKV // 2)
                ]  # [comp(hi,lo), sub-in-pair, qcol]
                Oaccs = [
                    ab.tile([P, NT, 512], FP, tag=f"Oacc{ci}", name=f"Oacc{ci}")
                    for ci in range(2)
                ]
                Ops = [
                    ab.tile([P, 2, NT, 512], F8, tag=f"Op{ci}", name=f"Op{ci}")
                    for ci in range(2)
                ]  # (hi, lo)
                rsbs = [
                    ab.tile([P, 512], FP, tag=f"rsb{ci}", name=f"rsb{ci}")
                    for ci in range(2)
                ]
                rs_ps = pp_rs.tile([P, H], FP, tag="rs")

                ef_cur = [None]
                pend = [None]  # delayed pair decompose closure

                def scores_s(s):
                    lo_s = LO16[s] * P
                    lo_p = LOP16[s] * P
                    if s % 2 == 0:
                        ef_cur[0] = efp.tile([P, 2, H], BF, tag="ef", name=f"ef{s}")
                    ef = ef_cur[0]
                    chs = _chunks512(lo_p, H)
                    pss = [
                        pp.tile([P, ce - cs], FP, tag="ps", name=f"pss{s}_{cs}")
                        for cs, ce in chs
                    ]
                    # ct-outer so each stationary kT slice loads once
                    nop = 0
                    for t in range(NT // 2):
                        for ps, (cs, ce) in zip(pss, chs):
                            mlo = max(cs, lo_s)
                            nc.tensor.matmul(
                                ps[:, mlo - cs : ce - cs],
                                lhsT=kT[:, 1, 2 * t : 2 * t + 2, s * P : (s + 1) * P],
                                rhs=qT[:, 0, 2 * t : 2 * t + 2, mlo:ce],
                                start=(nop < len(chs)),
                                stop=False,
                                perf_mode=DR,
                                skip_group_check=True,
                            )
                            nop += 1
                    for ct in range(NT):
                        for ps, (cs, ce) in zip(pss, chs):
                            mlo = max(cs, lo_s)
                            nc.tensor.matmul(
                                ps[:, mlo - cs : ce - cs],
                                lhsT=kT[:, 0:2, ct, s * P : (s + 1) * P],
                                rhs=qT[:, 0:2, ct, mlo:ce],
                                start=False,
                                stop=(ct == NT - 1),
                                perf_mode=DR,
                                skip_group_check=True,
                            )
                    for ps, (cs, ce) in zip(pss, chs):
                        for blk, mi in MASKS16[s]:
                            a = blk * P
                            if cs <= a < ce:
                                nc.vector.tensor_add(
                                    ps[:, a - cs : a - cs + P],
                                    ps[:, a - cs : a - cs + P],
                                    masks[:, mi, :],
                                )
                    if pend[0] is not None:
                        pend[0]()
                        pend[0] = None
                    # dead sliver [lo_p, lo_s): set to -1e9 on Act (scale=0
                    # kills the garbage psum) so exp = 0 there and the fp8
                    # pair reads as exact zeros for the paired attv ops
                    if lo_s > lo_p:
                        nc.scalar.activation(
                            pss[0][:, 0 : lo_s - lo_p],
                            pss[0][:, 0 : lo_s - lo_p],
                            AF.Identity,
                            bias=bias_sb[:, NT + 1 : NT + 2],
                            scale=0.0,
                        )
                    for ps, (cs, ce) in zip(pss, chs):
                        segs = (
                            [(cs, P, bqk_sb2), (P, ce, bqk_sb)]
                            if (s < 4 and cs == 0)
                            else [(cs, ce, bqk_sb)]
                        )
                        for g0, g1, bq_t in segs:
                            nc.scalar.activation(
                                ef[:, s % 2, g0:g1],
                                ps[:, g0 - cs : g1 - cs],
                                AF.Exp,
                                scale=1.0 / 32768.0,
                                bias=bq_t[:, s : s + 1],
                            )
                    if s % 2 == 1:
                        pend[0] = lambda s=s, ef=ef, lo_p=lo_p: _decompose(s, ef, lo_p)

                def _decompose(s, ef, lo_p):
                    if True:
                        # pair complete: decompose exp into the AT fp8 pair
                        # (hi copy on Pool, lo sub on DVE -- splits the ~28us
                        # of decompose across the two engines with slack)
                        atp = ATp[s // 2]
                        nc.gpsimd.tensor_copy(
                            atp[:, 0, :, lo_p:H], ef[:, :, lo_p:H]
                        )
                        nc.vector.tensor_sub(
                            atp[:, 1, :, lo_p:H],
                            ef[:, :, lo_p:H],
                            atp[:, 0, :, lo_p:H],
                        )
                        # rowsum for both subtiles of the pair
                        for sj in (s - 1, s):
                            lo_j = LOP16[sj] * P
                            for cs, ce in _chunks512(lo_j, H):
                                nc.tensor.matmul(
                                    rs_ps[:, cs:ce],
                                    lhsT=ones_sb[:],
                                    rhs=atp[:, 0:2, sj % 2, cs:ce],
                                    start=(sj == 0),
                                    stop=(
                                        (cs < 512 and sj == 7)
                                        or (cs >= 512 and sj == NKV - 1)
                                    ),
                                    perf_mode=DR,
                                    skip_group_check=True,
                                )

                def attv_chunk(ci, pairs):
                    cs, ce = ci * 512, (ci + 1) * 512
                    for ot in range(NT):
                        osl = slice(ot * P, (ot + 1) * P)
                        ps = pp.tile([P, 512], FP, tag="ps")
                        nops = 3 * len(pairs)
                        nop = 0
                        for p in pairs:
                            plo = max(cs, LOP16[2 * p] * P)
                            nc.tensor.matmul(
                                ps[:, plo - cs : 512],
                                lhsT=V[:, 1, 2 * p : 2 * p + 2, osl],
                                rhs=ATp[p][:, 0, :, plo:ce],
                                start=(nop == 0),
                                stop=False,
                                perf_mode=DR,
                                skip_group_check=True,
                            )
                            nop += 1
                            for sj in (0, 1):
                                nop += 1
                                nc.tensor.matmul(
                                    ps[:, plo - cs : 512],
                                    lhsT=V[:, 0:2, 2 * p + sj, osl],
                                    rhs=ATp[p][:, 0:2, sj, plo:ce],
                                    start=False,
                                    stop=(nop == nops),
                                    perf_mode=DR,
                                    skip_group_check=True,
                                )
                        nc.vector.tensor_copy(Oaccs[ci][:, ot, :], ps[:])

                def normalize_chunk(ci):
                    cs, ce = ci * 512, (ci + 1) * 512
                    Oc, Op, rsb = Oaccs[ci], Ops[ci], rsbs[ci]
                    nc.vector.tensor_copy(rsb[:], rs_ps[:, cs:ce])
                    nc.vector.reciprocal(rsb[:], rsb[:])
                    for ot in range(NT):
                        nc.vector.tensor_mul(Oc[:, ot, :], Oc[:, ot, :], rsb[:])
                        nc.scalar.activation(
                            Op[:, 0, ot, :], Oc[:, ot, :], AF.Identity
                        )
                        nc.vector.tensor_sub(
                            Op[:, 1, ot, :], Oc[:, ot, :], Op[:, 0, ot, :]
                        )

                def oproj_chunk(ci, evac):
                    Op = Ops[ci]
                    for o2 in range(NT):
                        osl = slice(o2 * P, (o2 + 1) * P)
                        ps = pp.tile([P, 512], FP, tag="ps", name=f"pso{ci}_{o2}")
                        first = True
                        for t in range(NT // 2):
                            nc.tensor.matmul(
                                ps[:],
                                lhsT=wp[:, 1, 2 * t : 2 * t + 2, osl],
                                rhs=Op[:, 0, 2 * t : 2 * t + 2, :],
                                start=first,
                                stop=False,
                                perf_mode=DR,
                            )
                            first = False
                        for ct in range(NT):
                            nc.tensor.matmul(
                                ps[:],
                                lhsT=wp[:, 0:2, ct, osl],
                                rhs=Op[:, 0:2, ct, :],
                                start=False,
                                stop=(ct == NT - 1),
                                perf_mode=DR,
                            )
                        ev = evac.tile([P, 512], BF, tag="evy", name=f"evy{ci}_{o2}")
                        nc.scalar.activation(
                            ev[:],
                            ps[:],
                            AF.Identity,
                            scale=1.0 / 1024.0,
                            bias=bias_sb[:, o2 : o2 + 1],
                        )
                        nc.sync.dma_start(
                            yT[(o2 * 2 + ci) * P : (o2 * 2 + ci + 1) * P, :],
                            ev[:],
                        )

                sc = tc.nc.named_scope("B1"); sc.__enter__()
                for s in range(16):
                    scores_s(s)
                if pend[0] is not None:
                    pend[0]()
                    pend[0] = None
                sc.__exit__(None, None, None)
                with tc.tile_pool(name="evac", bufs=3) as evac:
                    sc = tc.nc.named_scope("B2"); sc.__enter__()
                    attv_chunk(0, [0, 1, 2, 3])
                    normalize_chunk(0)
                    sc.__exit__(None, None, None)
                    sc = tc.nc.named_scope("D2"); sc.__enter__()
                    attv_chunk(1, [0, 1, 2, 3, 4, 5, 6, 7])
                    normalize_chunk(1)
                    sc.__exit__(None, None, None)
                    sc = tc.nc.named_scope("E"); sc.__enter__()
                    oproj_chunk(0, evac)
                    oproj_chunk(1, evac)
                    sc.__exit__(None, None, None)

    _split_waits(nc)
    return nc


_NC_CACHE = None


def _get_nc():
    global _NC_CACHE
    if _NC_CACHE is None:
        _NC_CACHE = _build_nc()
    return _NC_CACHE


def _pair(a, order="hl"):
    """Decompose fp32 array -> fp8 (hi, lo) or (lo, hi) pair along new axis 1.
    a: [P, ...]; returns [P, 2, ...] float8_e4m3."""
    import ml_dtypes

    a = np.asarray(a, dtype=np.float32)
    hi = a.astype(ml_dtypes.float8_e4m3)
    lo = (a - hi.astype(np.float32)).astype(ml_dtypes.float8_e4m3)
    pair = (hi, lo) if order == "hl" else (lo, hi)
    return np.ascontiguousarray(np.stack(pair, axis=1))


def _tile_major(m):
    """[C_in, N] -> [P, C_in//P, N] with partition dim first."""
    cin, n = m.shape
    return np.ascontiguousarray(m.reshape(cin // P, P, n).transpose(1, 0, 2))


def make_in_maps(x, w_qkv, b_qkv, w_proj, b_proj):
    """Host-side prep: shard + transpose + fp8-decompose inputs for 8 cores."""
    x = np.asarray(x, dtype=np.float32)
    w_qkv = np.asarray(w_qkv, dtype=np.float32)
    b_qkv = np.asarray(b_qkv, dtype=np.float32)
    w_proj = np.asarray(w_proj, dtype=np.float32)
    b_proj = np.asarray(b_proj, dtype=np.float32)
    import ml_dtypes

    s = 1.0 / np.sqrt(np.float32(C))

    # weights x32 so every fp8 operand sits at sigma >= 1 (far above the
    # e4m3 subnormal floor); the excess scale folds into Act scale args.
    # stored (lo, hi), layout [P cpart, 2, NT csub, C out]
    wq = _pair(_tile_major(w_qkv[0:C].T * 32.0), "lh")
    wk = _pair(_tile_major(w_qkv[C : 2 * C].T * 32.0), "lh")
    # per-ot contiguous blocks so each wk DMA has 2KB descriptor runs
    wk = np.ascontiguousarray(
        np.moveaxis(wk.reshape(P, 2, NT, NT, P), 3, 0)
    )
    wv = _pair(_tile_major(w_qkv[2 * C : 3 * C].T * 32.0), "lh")
    wp = _pair(_tile_major(w_proj.T * 32.0), "lh")
    bqp = _pair(_tile_major(b_qkv[0:C].reshape(C, 1) * 1024.0), "hl")

    bv = b_qkv[2 * C : 3 * C]
    beff = (b_proj + w_proj @ bv).reshape(NT, P).T
    bias = np.concatenate(
        [
            beff,
            np.full((P, 1), -np.log(ASCALE), np.float32),
            np.full((P, 1), NEG, np.float32),
            np.full((P, 1), -np.log(4.0), np.float32),
        ],
        axis=1,
    ).astype(np.float32)

    ones = np.ones((P, 2, P), dtype=np.float32).astype(ml_dtypes.float8_e4m3)

    # S^T mask tiles: partition = kv j (within subtile), free = query i
    triu = np.triu(np.ones((P, P), dtype=np.float32))
    trilm = np.where(triu > 0, 0.0, NEG).astype(np.float32)
    zeros = np.zeros((P, P), dtype=np.float32)
    negs = np.full((P, P), NEG, dtype=np.float32)

    shared = dict(
        wq_in=wq, wk_in=wk, wv_in=wv, wp_in=wp, bqp_in=bqp,
        bias_in=bias, ones_in=ones,
    )
    in_maps = []
    for core in range(8):
        b, h = core // 2, core % 2
        xb = x[b]  # [T, C]
        qrows = np.concatenate(
            [xb[(2 * bg + h) * 256 : (2 * bg + h + 1) * 256] for bg in range(4)],
            axis=0,
        )
        xo_pair = _pair(_tile_major(xb[0:H].T), "hl")
        m = np.stack(
            [
                trilm if h == 0 else zeros,   # m1d
                negs if h == 0 else zeros,    # m1f
                negs if h == 0 else trilm,    # m2d
            ],
            axis=1,
        )
        in_maps.append(
            dict(
                shared,
                xq_in=_pair(_tile_major(qrows.T), "hl"),
                xoA_in=np.ascontiguousarray(xo_pair[:, :, :, 0:256]),
                xoB_in=np.ascontiguousarray(xo_pair[:, :, :, 256:512]),
                xoC_in=np.ascontiguousarray(xo_pair[:, :, :, 512:1024]),
                xx_in=_pair(_tile_major(xb[H : 2 * H].T), "hl"),
                masks_in=np.ascontiguousarray(m),
            )
        )
    return in_maps


def assemble_output(results):
    B = 4
    y = np.empty((B, 2 * H, C), dtype=np.float32)
    for core in range(8):
        b, h = core // 2, core % 2
        yt = results[core]["yT"].astype(np.float32).reshape(NT, 2, P, 512)
        blk = yt.transpose(1, 3, 0, 2).reshape(H, C)
        blk4 = blk.reshape(4, 256, C)
        for bg in range(4):
            g = 2 * bg + h
            y[b, g * 256 : (g + 1) * 256, :] = blk4[bg]
    return y


def kernel(x, w_qkv, b_qkv, w_proj, b_proj):
    from concourse.bass_utils import run_bass_kernel_spmd

    nc = _get_nc()
    in_maps = make_in_maps(x, w_qkv, b_qkv, w_proj, b_proj)
    res = run_bass_kernel_spmd(nc, in_maps, list(range(8)))
    return assemble_output(res.results)
